# revision 50
# baseline (speedup 1.0000x reference)
# Distributed Bass kernel: causal multi-head attention block on 8 TRN2 NeuronCores.
#
# Problem (hardcoded): x [2, 4096, 768] f32, 12 heads x 64 dim, causal attention,
#   out = softmax(mask(q k^T / 8)) v  projected by Wo, all nn.Linear with bias.
#
# Sharding: core c -> batch b = c // 4, head-group hg = c % 4 (3 heads each).
#   Per core: QKV for its 3 heads over the full sequence (tensor parallel on
#   heads), flash-style causal attention, then 8 chunked AllGathers of preout^T
#   (bf16, [192, 512] per rank -> [768, 512]) within each 4-core batch group --
#   pipelined behind attention -- then an output projection sharded over dout
#   (each core computes its own 192 output columns for the full sequence,
#   written transposed [192, 4096] and flipped on the host).
#
# v3 changes vs v2:
#   - q/k projection runs in fp8-e4m3 DoubleRow (contract 256/instr): host
#     sends an extra fp8 copy of x^T and of [Wq;Wk]^T; halves those matmuls.
#   - a@v runs in fp8-e4m3 DoubleRow: the exp'd attention tile aT [128,2,512]
#     already holds two adjacent sj-chunks as free-dim halves, which is
#     exactly DoubleRow's operand layout; v is stored as per-pair two-chunk
#     stationaries [128,2,128] (64 v cols + 64 ones cols -> the softmax
#     denominator lands replicated on po partitions 64-127). Halves a@v.
#   - exp writes e4m3 directly (ACT) for off-diagonal pairs; diagonal pairs
#     go ACT->bf16 then mask-multiply->e4m3 (the mask also zeroes the stale
#     region half 1 doesn't compute, which DoubleRow's shared col range
#     would otherwise consume). A tunable share of off-diagonal pairs is
#     offloaded to the DVE as a Schraudolph exp: round(lg*A+B) written as
#     uint8 IS the e4m3 bit pattern of exp(lg/8) (approx err ~ e4m3 ulp).
#   - v-proj / qk / wo-proj stay bf16 (fp8 there costs too much accuracy:
#     v and wo errors hit the output linearly; qk contract is only 64 so
#     DoubleRow buys nothing).

import os

import numpy as np

B = 2
S = 4096
D = 768
HD = 64
NH = 12
NCORES = 8
HL = 3            # heads per core
DL = HL * HD      # 192: local q/k/v dims per core
SUP = 512         # si superchunk
NSUP = S // SUP   # 8
NKC = S // 128    # 32 sj chunks
NPAIR = NKC // 2  # 16 sj chunk-pairs
NDC = D // 128    # 6 contraction chunks
GROUPS = [[0, 1, 2, 3], [4, 5, 6, 7]]

_CACHE = {}


def _build_nc():
    import concourse.mybir as mybir
    from concourse import bacc
    from concourse.tile import TileContext

    f32 = mybir.dt.float32
    bf16 = mybir.dt.bfloat16
    fp8 = mybir.dt.float8e4
    u8 = mybir.dt.uint8
    EXP = mybir.ActivationFunctionType.Exp
    DR = mybir.MatmulPerfMode.DoubleRow

    nc = bacc.Bacc(num_devices=NCORES)

    qk8 = os.environ.get("QK8", "0") == "1"  # q/k projection in fp8 DoubleRow
    xT_p = nc.declare_dram_parameter("xT", [D, S], bf16, isOutput=False)
    if qk8:
        xT8_p = nc.declare_dram_parameter("xT8", [D, S], fp8, isOutput=False)
        wqk8_p = nc.declare_dram_parameter("wqk8", [D, 2 * DL], fp8, isOutput=False)
    else:
        wqk_p = nc.declare_dram_parameter("wqk", [D, 2 * DL], bf16, isOutput=False)
    bqk_p = nc.declare_dram_parameter("bqk", [2 * DL, 1], f32, isOutput=False)
    wv_p = nc.declare_dram_parameter("wv", [D, DL], bf16, isOutput=False)
    bv_p = nc.declare_dram_parameter("bv", [DL, 1], f32, isOutput=False)
    wo_p = nc.declare_dram_parameter("wo", [D, DL], bf16, isOutput=False)
    bo_p = nc.declare_dram_parameter("bo", [DL, 1], f32, isOutput=False)
    out_p = nc.declare_dram_parameter("out", [DL, S], f32, isOutput=True)
    DEBUG = os.environ.get("KDEBUG", "0") == "1"
    if DEBUG:
        dbgq_p = nc.declare_dram_parameter("dbgq", [64, HL * S], bf16, isOutput=True)
        dbgk_p = nc.declare_dram_parameter("dbgk", [64, HL * S], bf16, isOutput=True)
        dbgpo_p = nc.declare_dram_parameter("dbgpo", [64, HL * S], bf16, isOutput=True)
        # aT dumps for tasks (0,h,0) and (0,h,1): [128, h, pr, 2, 512]
        dbga_p = nc.declare_dram_parameter("dbga", [128, HL * 2 * 2 * 512], fp8, isOutput=True)
        dbgd_p = nc.declare_dram_parameter("dbgd", [128, HL * 512], f32, isOutput=True)  # po pre-normalize t=0

    NCHUNK = 8
    CW = S // NCHUNK  # 512 columns per AllGather chunk
    cins = [nc.dram_tensor(f"cc_in{c}", [DL, CW], bf16) for c in range(NCHUNK)]
    couts = [nc.dram_tensor(f"cc_out{c}", [D, CW], bf16) for c in range(NCHUNK)]
    # the last chunk's AllGather is split per head so each 1/3 fires as soon
    # as that head's normalize completes (shrinks the end-of-kernel tail)
    cins7 = [nc.dram_tensor(f"cc_in7h{h}", [HD, CW], bf16) for h in range(HL)]
    couts7 = [nc.dram_tensor(f"cc_out7h{h}", [4 * HD, CW], bf16) for h in range(HL)]

    # Schraudolph exp -> e4m3 bit pattern: round(lg * A + B) as uint8.
    # e4m3 (bias 7, 3 mantissa bits): pattern(v) ~ 8*(log2 v + 7);
    # v = exp(lg/8) -> pattern = lg/(8 ln2)*8 + 56 - sigma.
    EXP_A = 1.0 / float(np.log(2.0))
    EXP_B = 56.0 - 0.47
    # 1-in-N off-diagonal exp pairs go to the DVE (0 = all on ACT)
    dve_k = int(os.environ.get("DVE_K", "6"))  # of every 16 nondiag pairs
    av8 = os.environ.get("AV8", "1") == "1"  # a@v in fp8 DoubleRow vs bf16
    # superchunks t < T0 keep the bf16 a@v path: their outputs average few
    # attention terms, so quantization error is amplified there (t=0 alone
    # carries ~35% of the output norm); t >= T0 in fp8 costs little error
    # and carries ~93% of the a@v matmul columns.
    T0 = int(os.environ.get("T0", "2")) if av8 else NSUP
    # diagnostics: round v / a through fp8 inside the bf16 path
    diag_v8 = os.environ.get("DIAG_V8", "0") == "1"
    diag_a8 = os.environ.get("DIAG_A8", "0") == "1"

    with TileContext(nc) as tc:
        with (
            tc.tile_pool(name="const", bufs=1) as cpool,
            tc.tile_pool(name="at", bufs=4) as atpool,
            tc.tile_pool(name="atd", bufs=2) as atdpool,
            tc.tile_pool(name="ps", bufs=2) as pspool,
            tc.tile_pool(name="ot", bufs=2 if DEBUG else 4) as otpool,
            tc.tile_pool(name="ccp", bufs=1) as ccpool,
        ):
            # ---------------- constants / weights ----------------
            # DMA order matters: the first projection matmul needs wqk + the
            # first x^T superchunk, so those go to the queue first.
            if qk8:
                wqk8_sb = cpool.tile([128, NDC, 2 * DL], fp8, name="wqk8_sb")
                nc.sync.dma_start(
                    out=wqk8_sb[:, :, :],
                    in_=wqk8_p[:, :].rearrange("(c p) m -> p c m", p=128),
                )
                xT8 = cpool.tile([128, NDC, S], fp8, name="xT8")  # 24KB/partition
                nc.sync.dma_start(
                    out=xT8[:, :, 0:SUP],
                    in_=xT8_p[:, :].rearrange("(c p) s -> p c s", p=128)[:, :, 0:SUP],
                )
            else:
                wqk_sb = cpool.tile([128, NDC, 2 * DL], bf16, name="wqk_sb")
                nc.sync.dma_start(
                    out=wqk_sb[:, :, :],
                    in_=wqk_p[:, :].rearrange("(c p) m -> p c m", p=128),
                )
            wv_sb = cpool.tile([128, NDC, DL], bf16, name="wv_sb")
            nc.sync.dma_start(
                out=wv_sb[:, :, :],
                in_=wv_p[:, :].rearrange("(c p) m -> p c m", p=128),
            )
            xT = cpool.tile([128, NDC, S], bf16, name="xT")  # 48KB/partition
            nc.sync.dma_start(
                out=xT[:, :, 0:SUP],
                in_=xT_p[:, :].rearrange("(c p) s -> p c s", p=128)[:, :, 0:SUP],
            )
            bqk_sb = cpool.tile([128, 2 * DL // 128, 1], f32, name="bqk_sb")
            nc.sync.dma_start(
                out=bqk_sb[:, :, :], in_=bqk_p[:, :].rearrange("(c p) o -> p c o", p=128)
            )
            wo_sb = cpool.tile([128, NDC, DL], bf16, name="wo_sb")
            nc.sync.dma_start(
                out=wo_sb[:, :, :],
                in_=wo_p[:, :].rearrange("(c p) m -> p c m", p=128),
            )
            bv_sb = cpool.tile([64, HL, 1], f32, name="bv_sb")
            nc.sync.dma_start(
                out=bv_sb[:, :, :], in_=bv_p[:, :].rearrange("(h p) o -> p h o", p=64)
            )
            bo0_sb = cpool.tile([128, 1], f32, name="bo0_sb")
            nc.sync.dma_start(out=bo0_sb[:, :], in_=bo_p[0:128, :])
            bo1_sb = cpool.tile([64, 1], f32, name="bo1_sb")
            nc.sync.dma_start(out=bo1_sb[:, :], in_=bo_p[128:DL, :])

            # multiplicative causal masks for the 4 diagonal sj-chunk offsets:
            # masks[p, k, f] = 1.0 if (f - p - 128k) >= 0 else 0.0
            masks = cpool.tile([128, 4, SUP], bf16, name="masks")
            nc.gpsimd.memset(masks[:, :, :], 1.0)
            for k in range(4):
                nc.gpsimd.affine_select(
                    out=masks[:, k, :],
                    in_=masks[:, k, :],
                    compare_op=mybir.AluOpType.is_ge,
                    fill=0.0,
                    base=-128 * k,
                    pattern=[[1, SUP]],
                    channel_multiplier=-1,
                )

            # contract-128 zero-padded: real data at partitions 0-63,
            # zeros at 64-127 (k side; q upper half zeroed too so no
            # NaN garbage enters the array).
            qT = cpool.tile([128, HL, S], bf16, name="qT")
            kT = cpool.tile([128, HL, S], bf16, name="kT")
            nc.gpsimd.memset(kT[64:128, :, :], 0.0)
            nc.gpsimd.memset(qT[64:128, :, :], 0.0)

            # v stored per sj chunk-PAIR as DoubleRow stationaries:
            # v2[:, pr, h, i, 0:64] = v of chunk j=2pr+i, cols 64:128 = ones
            # -- the a@v DoubleRow matmul then emits the softmax denominator
            # REPLICATED on po partitions 64-127 (no copies needed for the
            # normalize).
            # v for the fp8 DoubleRow path (t >= T0): per-pair two-chunk
            # stationaries, cols 0-63 the values, 64-127 ones (the a@v matmul
            # then emits the softmax denominator replicated on po rows 64-127)
            if T0 < NSUP:
                v2 = cpool.tile([128, NPAIR, HL, 2, 128], fp8, name="v2")
                nc.vector.memset(v2[:, :, :, :, :], 1.0)
            # v for the bf16 path (t < T0, sj chunks 0..4*T0-1): 64 values +
            # 32 ones cols (denominator replicated on po rows 64-95)
            NJB = 4 * max(T0, 1)
            vb = cpool.tile([128, NJB, HL, 96], bf16, name="vb")
            nc.vector.memset(vb[:, :, :, :], 1.0)
            poT = cpool.tile([64, HL, S], bf16, name="poT")  # preout^T, per head

            # ---------------- phase 1: x^T DMA + qk/v projections ----------------
            mmctx = tc.tile_pool(name="mm", bufs=4, space="PSUM")
            mmpsum = mmctx.__enter__()
            for t in range(NSUP):
                if t > 0:
                    if qk8:
                        nc.sync.dma_start(
                            out=xT8[:, :, t * SUP : (t + 1) * SUP],
                            in_=xT8_p[:, :].rearrange("(c p) s -> p c s", p=128)[
                                :, :, t * SUP : (t + 1) * SUP
                            ],
                        )
                    nc.sync.dma_start(
                        out=xT[:, :, t * SUP : (t + 1) * SUP],
                        in_=xT_p[:, :].rearrange("(c p) s -> p c s", p=128)[
                            :, :, t * SUP : (t + 1) * SUP
                        ],
                    )

                # q/k projection for this superchunk: out [m, s]. fp8 DoubleRow
                # (contract 256/instr) when qk8, else bf16 contract-128.
                for mc in range(2 * DL // 128):
                    ps = mmpsum.tile([128, 512], f32, name="ps", tag="mm")
                    if qk8:
                        for c in range(NDC // 2):
                            nc.tensor.matmul(
                                ps[:, :],
                                lhsT=wqk8_sb[:, 2 * c : 2 * c + 2, mc * 128 : (mc + 1) * 128],
                                rhs=xT8[:, 2 * c : 2 * c + 2, t * SUP : (t + 1) * SUP],
                                start=(c == 0),
                                stop=(c == NDC // 2 - 1),
                                perf_mode=DR,
                            )
                    else:
                        for dc in range(NDC):
                            nc.tensor.matmul(
                                ps[:, :],
                                lhsT=wqk_sb[:, dc, mc * 128 : (mc + 1) * 128],
                                rhs=xT[:, dc, t * SUP : (t + 1) * SUP],
                                start=(dc == 0),
                                stop=(dc == NDC - 1),
                            )
                    # bias adds on the ACT engine (out = Copy(in + bias)): it
                    # is idle during phase 1 and the DVE needs its headroom
                    # for the exp share in phase 2
                    COPYF = mybir.ActivationFunctionType.Identity
                    for half in (0, 1):
                        g = mc * 128 + half * 64  # global row in [q(192); k(192)]
                        src = ps[half * 64 : half * 64 + 64, :]
                        bias = bqk_sb[half * 64 : half * 64 + 64, mc, :]
                        if g < DL:
                            h = g // 64
                            nc.scalar.activation(
                                qT[0:64, h, t * SUP : (t + 1) * SUP], src,
                                COPYF, bias=bias,
                            )
                        else:
                            h = (g - DL) // 64
                            nc.scalar.activation(
                                kT[0:64, h, t * SUP : (t + 1) * SUP], src,
                                COPYF, bias=bias,
                            )

                # v for this superchunk: out [s, m] (bias deferred to post-softmax)
                for sub in range(4):
                    j = t * 4 + sub
                    pv = mmpsum.tile([128, 512], f32, name="pv", tag="mm")
                    for dc in range(NDC):
                        nc.tensor.matmul(
                            pv[:, 0:DL],
                            lhsT=xT[:, dc, j * 128 : (j + 1) * 128],
                            rhs=wv_sb[:, dc, :],
                            start=(dc == 0),
                            stop=(dc == NDC - 1),
                        )
                    pvr = pv[:, 0:DL].rearrange("p (h w) -> p h w", h=HL)
                    if T0 < NSUP:
                        if diag_v8:
                            v8s = pspool.tile([128, HL, HD], fp8, name="v8s", tag="v8s")
                            nc.scalar.copy(v8s[:, :, :], pvr)
                            nc.scalar.copy(v2[:, j // 2, :, j % 2, 0:HD], v8s[:, :, :])
                        else:
                            nc.scalar.copy(v2[:, j // 2, :, j % 2, 0:HD], pvr)
                    if j < NJB:
                        nc.scalar.copy(vb[:, j, :, 0:HD], pvr)

            # ---------------- phase 2: flash attention (logits transposed) ----------------
            # Software-pipelined emission: the PE executes its queue in order,
            # so qk of pair k+1 is emitted BEFORE a@v of pair k -- the qk
            # matmuls then run while the scalar engine computes exp(pair k),
            # instead of the PE idling behind exp each pair.
            mmctx.__exit__(None, None, None)
            lgctx = tc.tile_pool(name="lg", bufs=3, space="PSUM")
            lgpsum = lgctx.__enter__()
            poctx = tc.tile_pool(name="po", bufs=2, space="PSUM")
            popsum = poctx.__enter__()
            tasks = [
                (t, h, pr)
                for t in range(NSUP)
                for h in range(HL)
                for pr in range(2 * t + 2)
            ]
            if DEBUG:
                dbga_sb = cpool.tile([128, HL, 2, 2, 512], fp8, name="dbga_sb")
                dbgd_sb = cpool.tile([128, HL, 512], f32, name="dbgd_sb")

            def emit_qk(task):
                t, h, pr = task
                si0 = t * SUP
                # exact per-chunk causal offsets (cols < 128*krel fully masked)
                offs = [max(0, (2 * pr + half - 4 * t) * 128) for half in (0, 1)]
                off = min(offs)  # exp + a@v range
                diag = pr >= 2 * t  # last two pairs: contain diagonal chunks
                lg = lgpsum.tile([128, 2, 512], f32, name="lg", tag="lg")
                for half in (0, 1):
                    j = 2 * pr + half
                    # diag pairs: compute half 1 from `off` too -- exp reads
                    # [off:] and stale PSUM there could blow up to inf (the
                    # extra cols are real logits, zeroed by the mask below)
                    o = off if diag else offs[half]
                    sj0 = 128 * j
                    nc.tensor.matmul(
                        lg[:, half, o:],
                        lhsT=kT[:, h, sj0 : sj0 + 128],
                        rhs=qT[:, h, si0 + o : si0 + SUP],
                        start=True,
                        stop=True,
                    )
                use8 = t >= T0
                adt = fp8 if use8 else bf16
                aT = atpool.tile(
                    [128, 2, 512], adt, name="aT",
                    tag="at8" if use8 else "atb", bufs=4 if use8 else 3,
                )
                if diag:
                    # exp -> bf16 scratch, then per-half causal-mask multiply
                    # (also zeroes half 1's stale cols [off:offs[1]))
                    aTd = atdpool.tile([128, 2, 512], bf16, name="aTd", tag="atd")
                    nc.scalar.activation(
                        aTd[:, :, off:], lg[:, :, off:], EXP, scale=0.125
                    )
                    for half in (0, 1):
                        krel = 2 * pr + half - 4 * t
                        nc.vector.tensor_mul(
                            aT[:, half, off:],
                            aTd[:, half, off:],
                            masks[:, krel, off:],
                        )
                elif use8 and dve_k and (t * 31 + h * 7 + pr) % 16 < dve_k:
                    # Schraudolph exp on the DVE, straight to the e4m3 pattern
                    nc.vector.tensor_scalar(
                        out=aT[:, :, :].bitcast(u8),
                        in0=lg[:, :, :],
                        scalar1=EXP_A,
                        scalar2=EXP_B,
                        op0=mybir.AluOpType.mult,
                        op1=mybir.AluOpType.add,
                    )
                elif diag_a8:
                    a8s = atdpool.tile([128, 2, 512], fp8, name="a8s", tag="a8s")
                    nc.scalar.activation(a8s[:, :, :], lg[:, :, :], EXP, scale=0.125)
                    nc.vector.tensor_copy(aT[:, :, :], a8s[:, :, :])
                else:
                    # exp of both halves in one ACT instruction
                    nc.scalar.activation(aT[:, :, :], lg[:, :, :], EXP, scale=0.125)
                if DEBUG and t == 0:
                    nc.vector.tensor_copy(dbga_sb[:, h, pr, :, off:], aT[:, :, off:])
                return aT, off

            po_tiles = {}

            def emit_av(task, aT, off):
                t, h, pr = task
                if pr == 0:
                    po_tiles[(t, h)] = popsum.tile([128, 512], f32, name="po", tag="po")
                po = po_tiles[(t, h)]
                if t >= T0:
                    # one DoubleRow matmul covers both sj chunks of the pair
                    # (contract 256: 128 partitions x 2 free halves)
                    nc.tensor.matmul(
                        po[:, off:],
                        lhsT=v2[:, pr, h, :, :],
                        rhs=aT[:, :, off:],
                        start=(pr == 0),
                        stop=(pr == 2 * t + 1),
                        perf_mode=DR,
                    )
                else:
                    diag = pr >= 2 * t
                    for half in (0, 1):
                        o = off + 128 * half if diag else 0
                        nc.tensor.matmul(
                            po[0:96, o:],
                            lhsT=vb[:, 2 * pr + half, h, :],
                            rhs=aT[:, half, o:],
                            start=(pr == 0 and half == 0),
                            stop=(pr == 2 * t + 1 and half == 1),
                        )
                if pr < 2 * t + 1:
                    return
                if DEBUG and t == int(os.environ.get("DBG_T", "0")):
                    nc.vector.tensor_copy(dbgd_sb[:, h, :], po[:, :])
                # last pair of (t, h): normalize by the replicated softmax
                # denominator (po rows 64-127) + deferred v bias. Pure DVE.
                si0 = t * SUP
                bcs = pspool.tile([64, 512], f32, name="bcs", tag="bc")
                if t >= T0:
                    nc.vector.tensor_copy(bcs[:, :], po[64:128, :])
                else:
                    nc.vector.tensor_copy(bcs[0:32, :], po[64:96, :])
                    nc.vector.tensor_copy(bcs[32:64, :], po[64:96, :])
                nc.vector.reciprocal_approx_fast(out=bcs[:, :], in_=bcs[:, :])
                nc.vector.tensor_mul(
                    poT[:, h, si0 : si0 + SUP], po[0:64, :], bcs[:, :]
                )
                nc.vector.tensor_scalar_add(
                    poT[:, h, si0 : si0 + SUP],
                    poT[:, h, si0 : si0 + SUP],
                    bv_sb[:, h, :],
                )
                del po_tiles[(t, h)]
                if t == NSUP - 1:
                    # final chunk: per-head AllGather, fired immediately
                    nc.sync.dma_start(
                        out=cins7[h][:, :], in_=poT[:, h, t * CW : (t + 1) * CW]
                    )
                    nc.gpsimd.collective_compute(
                        "AllGather",
                        mybir.AluOpType.bypass,
                        replica_groups=GROUPS,
                        ins=[cins7[h][:, :]],
                        outs=[couts7[h][:, :]],
                    )
                    return
                if h < HL - 1:
                    return
                # ---------------- phase 3: chunked AllGather ----------------
                c = t
                for hh in range(HL):
                    nc.sync.dma_start(
                        out=cins[c][HD * hh : HD * (hh + 1), :],
                        in_=poT[:, hh, c * CW : (c + 1) * CW],
                    )
                nc.gpsimd.collective_compute(
                    "AllGather",
                    mybir.AluOpType.bypass,
                    replica_groups=GROUPS,
                    ins=[cins[c][:, :]],
                    outs=[couts[c][:, :]],
                )

            # ---------------- phase 4: output projection (dout-sharded) ----------------
            strips_of = {}

            def emit_strips(c):
                strips = []
                for dc in range(NDC):
                    strip = ccpool.tile(
                        [128, CW], bf16, name=f"ccs{c}_{dc}", tag=f"ccs{dc}",
                        bufs=2 if DEBUG else 3,
                    )
                    if c == NCHUNK - 1:
                        for half in (0, 1):
                            r, hh = divmod(2 * dc + half, HL)
                            nc.sync.dma_start(
                                out=strip[64 * half : 64 * half + 64, :],
                                in_=couts7[hh][64 * r : 64 * r + 64, :],
                            )
                    else:
                        nc.sync.dma_start(
                            out=strip[:, :], in_=couts[c][dc * 128 : (dc + 1) * 128, :]
                        )
                    strips.append(strip)
                strips_of[c] = strips

            def emit_oproj(c):
                strips = strips_of.pop(c)
                for oc, M0, bo_sb in ((0, 128, bo0_sb), (1, 64, bo1_sb)):
                    pso = ompsum.tile([128, 512], f32, name="pso", tag="om")
                    for dc in range(NDC):
                        nc.tensor.matmul(
                            pso[0:M0, :],
                            lhsT=wo_sb[:, dc, oc * 128 : oc * 128 + M0],
                            rhs=strips[dc][:, :],
                            start=(dc == 0),
                            stop=(dc == NDC - 1),
                        )
                    ot = otpool.tile([128, 512], f32, name="ot", tag="ot")
                    nc.vector.tensor_scalar_add(ot[0:M0, :], pso[0:M0, :], bo_sb[:, :])
                    nc.sync.dma_start(
                        out=out_p[oc * 128 : oc * 128 + M0, c * SUP : (c + 1) * SUP],
                        in_=ot[0:M0, :],
                    )

            prev = None
            for task in tasks:
                cur = (task, *emit_qk(task))
                if prev is not None:
                    emit_av(*prev)
                prev = cur
            emit_av(*prev)
            if DEBUG:
                nc.sync.dma_start(
                    out=dbgq_p[:, :], in_=qT[0:64, :, :].rearrange("p h s -> p (h s)")
                )
                nc.sync.dma_start(
                    out=dbgk_p[:, :], in_=kT[0:64, :, :].rearrange("p h s -> p (h s)")
                )
                nc.sync.dma_start(
                    out=dbgpo_p[:, :], in_=poT[:, :, :].rearrange("p h s -> p (h s)")
                )
                nc.sync.dma_start(
                    out=dbga_p[:, :],
                    in_=dbga_sb[:, :, :, :, :].rearrange("p h r i s -> p (h r i s)"),
                )
                nc.sync.dma_start(
                    out=dbgd_p[:, :], in_=dbgd_sb[:, :, :].rearrange("p h s -> p (h s)")
                )
            poctx.__exit__(None, None, None)
            lgctx.__exit__(None, None, None)
            omctx = tc.tile_pool(name="om", bufs=4, space="PSUM")
            ompsum = omctx.__enter__()
            for c in range(NCHUNK):
                emit_strips(c)
                emit_oproj(c)
            omctx.__exit__(None, None, None)

    nc.finalize()
    return nc


def _get_nc():
    if "nc" not in _CACHE:
        _CACHE["nc"] = _build_nc()
    return _CACHE["nc"]


def _make_in_maps(x, Wq_w, Wq_b, Wk_w, Wk_b, Wv_w, Wv_b, Wo_w, Wo_b):
    import ml_dtypes

    bf = ml_dtypes.bfloat16
    e4 = ml_dtypes.float8_e4m3
    f = np.float32
    qk8 = os.environ.get("QK8", "0") == "1"
    in_maps = []
    for c in range(NCORES):
        b, hg = divmod(c, 4)
        r = slice(hg * DL, (hg + 1) * DL)
        xTb = np.ascontiguousarray(x[b].T)
        wqkT = np.ascontiguousarray(np.concatenate([Wq_w[r], Wk_w[r]], axis=0).T)
        m = {
            "xT": xTb.astype(bf),
            "bqk": np.ascontiguousarray(
                np.concatenate([Wq_b[r], Wk_b[r]])[:, None], dtype=f
            ),
            "wv": np.ascontiguousarray(Wv_w[r].T.astype(bf)),
            "bv": np.ascontiguousarray(Wv_b[r][:, None], dtype=f),
            "wo": np.ascontiguousarray(Wo_w[r].T.astype(bf)),
            "bo": np.ascontiguousarray(Wo_b[r][:, None], dtype=f),
        }
        if qk8:
            m["xT8"] = xTb.astype(e4)
            m["wqk8"] = wqkT.astype(e4)
        else:
            m["wqk"] = wqkT.astype(bf)
        in_maps.append(m)
    return in_maps


def run_on_hw(in_maps, trace=False):
    from concourse.bass_utils import run_bass_kernel_spmd

    nc = _get_nc()
    return run_bass_kernel_spmd(nc, in_maps, core_ids=list(range(NCORES)), trace=trace)


def kernel(x, Wq_w, Wq_b, Wk_w, Wk_b, Wv_w, Wv_b, Wo_w, Wo_b):
    in_maps = _make_in_maps(
        np.asarray(x, dtype=np.float32),
        *[
            np.asarray(a, dtype=np.float32)
            for a in (Wq_w, Wq_b, Wk_w, Wk_b, Wv_w, Wv_b, Wo_w, Wo_b)
        ],
    )
    res = run_on_hw(in_maps, trace=False)
    out = np.empty((B, S, D), dtype=np.float32)
    for c in range(NCORES):
        b, hg = divmod(c, 4)
        out[b, :, hg * DL : (hg + 1) * DL] = res.results[c]["out"].T
    return out
